# revision 15
# baseline (speedup 1.0000x reference)
import sys

sys.path.insert(0, "/opt/trn_rl_repo")

import numpy as np

import concourse.bacc as bacc
import concourse.bass as bass
import concourse.mybir as mybir
from concourse.tile import TileContext
from concourse.bass_utils import run_bass_kernel_spmd

B, S, D = 4, 4096, 1024
N_CORES = 8
SS = S // N_CORES  # 512 seq rows per core
P = 128
NJ = SS // P  # 4 partition-blocks of seq per core
SCALE = 32.0  # sqrt(1024)

TWO_PI = float(np.float32(2.0 * np.pi))
INV_2PI = float(np.float32(1.0 / (2.0 * np.pi)))
PI_F = float(np.float32(np.pi))

_TRACE = False
_LAST_RESULTS = None


def _make_tab() -> np.ndarray:
    # [1, 2D]: per-column frequency 1/10000^(2i/D) and phase (pi/2 on odd
    # columns turns sin into cos), matching reference f32 arithmetic.
    i = np.arange(D, dtype=np.float32)
    expnt = (np.float32(2.0) * i / np.float32(D)).astype(np.float32)
    denom = np.power(np.float32(10000.0), expnt).astype(np.float32)
    freq = (np.float32(1.0) / denom).astype(np.float32)
    phase = np.where(
        np.arange(D) % 2 == 0, np.float32(0.0), np.float32(np.pi / 2)
    ).astype(np.float32)
    return np.concatenate([freq, phase])[None, :]


def _build() -> bass.Bass:
    nc = bacc.Bacc()
    f32 = mybir.dt.float32
    i32 = mybir.dt.int32
    emb = nc.declare_dram_parameter("emb", [B, SS, D], f32, isOutput=False)
    srow = nc.declare_dram_parameter("srow", [P, 1], f32, isOutput=False)
    tab = nc.declare_dram_parameter("tab", [1, 2 * D], f32, isOutput=False)
    out = nc.declare_dram_parameter("out", [B, SS, D], f32, isOutput=True)

    with TileContext(nc) as tc:
        with tc.tile_pool(name="pool", bufs=2) as pool:
            # ---- compute pe[512,1024] on-chip as [128, 4*1024] ----
            tab0 = pool.tile([1, 2 * D], f32, tag="tab0", bufs=1)
            nc.sync.dma_start(out=tab0[:], in_=tab[:])
            tabb = pool.tile([P, 2 * D], f32, tag="tabb", bufs=1)
            nc.gpsimd.partition_broadcast(tabb[:], tab0[:])

            # seq row for partition p, block j is c*SS + p*NJ + j ("(p j)" order
            # so DMA moves one 16KB contiguous run per partition)
            srow_t = pool.tile([P, 1], f32, tag="srow", bufs=1)
            nc.sync.dma_start(out=srow_t[:], in_=srow[:])
            svs = [srow_t]
            for j in range(1, NJ):
                sv = pool.tile([P, 1], f32, tag=f"sv{j}", name=f"sv{j}", bufs=1)
                nc.vector.tensor_scalar_add(
                    out=sv[:], in0=srow_t[:], scalar1=float(j)
                )
                svs.append(sv)

            ang = pool.tile([P, NJ * D], f32, tag="ang", bufs=1)
            q = pool.tile([P, NJ * D], f32, tag="q", bufs=1)
            ki = pool.tile([P, NJ * D], i32, tag="ki", bufs=1)
            kf = pool.tile([P, NJ * D], f32, tag="kf", bufs=1)
            red = pool.tile([P, NJ * D], f32, tag="red", bufs=1)
            pe_tile = pool.tile([P, NJ * D], f32, tag="pe", bufs=1)
            for j in range(NJ):
                sl = slice(j * D, (j + 1) * D)
                nc.vector.scalar_tensor_tensor(
                    out=ang[:, sl], in0=tabb[:, 0:D], scalar=svs[j][:],
                    in1=tabb[:, D : 2 * D],
                    op0=mybir.AluOpType.mult, op1=mybir.AluOpType.add,
                )
            # range-reduce to [-pi, pi]: k = int(ang/2pi + 0.5); red = ang - k*2pi
            nc.vector.tensor_scalar(
                out=q[:], in0=ang[:], scalar1=INV_2PI, scalar2=0.5,
                op0=mybir.AluOpType.mult, op1=mybir.AluOpType.add,
            )
            nc.vector.tensor_copy(out=ki[:], in_=q[:])
            nc.vector.tensor_copy(out=kf[:], in_=ki[:])
            nc.vector.scalar_tensor_tensor(
                out=red[:], in0=kf[:], scalar=-TWO_PI, in1=ang[:],
                op0=mybir.AluOpType.mult, op1=mybir.AluOpType.add,
            )
            nc.vector.add_range_wrap(
                out=red[:], in_=red[:], shift=0.0, bound=PI_F, period=TWO_PI
            )
            nc.scalar.activation(
                out=pe_tile[:], in_=red[:], func=mybir.ActivationFunctionType.Sin
            )

            # ---- out[b] = emb[b]*sqrt(D) + pe ----
            for b in range(B):
                t = pool.tile([P, NJ * D], f32, tag="io", name="t", bufs=3)
                nc.sync.dma_start(
                    out=t[:].rearrange("p (j d) -> p j d", d=D),
                    in_=emb[b].rearrange("(p j) d -> p j d", p=P),
                )
                nc.vector.scalar_tensor_tensor(
                    out=t[:],
                    in0=t[:],
                    scalar=SCALE,
                    in1=pe_tile[:],
                    op0=mybir.AluOpType.mult,
                    op1=mybir.AluOpType.add,
                )
                nc.sync.dma_start(
                    out=out[b].rearrange("(p j) d -> p j d", p=P),
                    in_=t[:].rearrange("p (j d) -> p j d", d=D),
                )
    nc.finalize()
    return nc


_CACHE: dict = {}


def kernel(embeddings: np.ndarray) -> np.ndarray:
    global _LAST_RESULTS
    emb = np.asarray(embeddings, dtype=np.float32)
    if "nc" not in _CACHE:
        _CACHE["nc"] = _build()
        _CACHE["tab"] = _make_tab()
    nc = _CACHE["nc"]
    tab = _CACHE["tab"]

    in_maps = []
    for c in range(N_CORES):
        sl = emb[:, c * SS : (c + 1) * SS, :]
        srow = (np.arange(P, dtype=np.float32) * NJ + c * SS)[:, None]
        in_maps.append(
            {
                "emb": np.ascontiguousarray(sl),
                "srow": srow,
                "tab": tab,
            }
        )
    res = run_bass_kernel_spmd(
        nc, in_maps, core_ids=list(range(N_CORES)), trace=_TRACE
    )
    _LAST_RESULTS = res
    return np.concatenate([r["out"] for r in res.results], axis=1)


# revision 16
# speedup vs baseline: 1.0325x; 1.0325x over previous
import sys

sys.path.insert(0, "/opt/trn_rl_repo")

import numpy as np

import concourse.bacc as bacc
import concourse.bass as bass
import concourse.mybir as mybir
from concourse.tile import TileContext
from concourse.bass_utils import run_bass_kernel_spmd

B, S, D = 4, 4096, 1024
N_CORES = 8
SS = S // N_CORES  # 512 seq rows per core
P = 128
NJ = SS // P  # 4 partition-blocks of seq per core
SCALE = 32.0  # sqrt(1024)

TWO_PI = float(np.float32(2.0 * np.pi))
INV_2PI = float(np.float32(1.0 / (2.0 * np.pi)))

_TRACE = False
_LAST_RESULTS = None


def _make_tab() -> np.ndarray:
    # [1, 2D]: per-column frequency 1/10000^(2i/D) and phase (pi/2 on odd
    # columns turns sin into cos), matching reference f32 arithmetic.
    i = np.arange(D, dtype=np.float32)
    expnt = (np.float32(2.0) * i / np.float32(D)).astype(np.float32)
    denom = np.power(np.float32(10000.0), expnt).astype(np.float32)
    freq = (np.float32(1.0) / denom).astype(np.float32)
    phase = np.where(
        np.arange(D) % 2 == 0, np.float32(0.0), np.float32(np.pi / 2)
    ).astype(np.float32)
    return np.concatenate([freq, phase])[None, :]


def _build() -> bass.Bass:
    nc = bacc.Bacc()
    f32 = mybir.dt.float32
    i32 = mybir.dt.int32
    emb = nc.declare_dram_parameter("emb", [B, SS, D], f32, isOutput=False)
    srow = nc.declare_dram_parameter("srow", [P, 1], f32, isOutput=False)
    tab = nc.declare_dram_parameter("tab", [1, 2 * D], f32, isOutput=False)
    out = nc.declare_dram_parameter("out", [B, SS, D], f32, isOutput=True)

    with TileContext(nc) as tc:
        with tc.tile_pool(name="pool", bufs=2) as pool:
            tab0 = pool.tile([1, 2 * D], f32, tag="tab0", bufs=1)
            nc.sync.dma_start(out=tab0[:], in_=tab[:])
            tabb = pool.tile([P, 2 * D], f32, tag="tabb", bufs=1)
            nc.gpsimd.partition_broadcast(tabb[:], tab0[:])

            srow_t = pool.tile([P, 1], f32, tag="srow", bufs=1)
            nc.sync.dma_start(out=srow_t[:], in_=srow[:])
            svs = [srow_t]
            for j in range(1, NJ):
                sv = pool.tile([P, 1], f32, tag=f"sv{j}", name=f"sv{j}", bufs=1)
                nc.vector.tensor_scalar_add(
                    out=sv[:], in0=srow_t[:], scalar1=float(j * P)
                )
                svs.append(sv)

            pe_tiles = []
            io_tiles = {}

            def io_block(b, j, pe_j):
                # out[b, jP:(j+1)P, :] = emb[b, jP:(j+1)P, :]*SCALE + pe_j
                t = pool.tile([P, D], f32, tag="io", name="t", bufs=8)
                nc.sync.dma_start(out=t[:], in_=emb[b][j * P : (j + 1) * P, :])
                nc.vector.scalar_tensor_tensor(
                    out=t[:], in0=t[:], scalar=SCALE, in1=pe_j[:],
                    op0=mybir.AluOpType.mult, op1=mybir.AluOpType.add,
                )
                nc.sync.dma_start(out=out[b][j * P : (j + 1) * P, :], in_=t[:])

            for j in range(NJ):
                # pe_j[p, d] = sin(s*freq + phase - round((s*freq+phase)/2pi)*2pi)
                ang = pool.tile([P, D], f32, tag=f"ang{j}", name="ang", bufs=1)
                ki = pool.tile([P, D], i32, tag=f"ki{j}", name="ki", bufs=1)
                red = pool.tile([P, D], f32, tag=f"red{j}", name="red", bufs=1)
                pe_j = pool.tile([P, D], f32, tag=f"pe{j}", name="pe_j", bufs=1)
                nc.vector.scalar_tensor_tensor(
                    out=ang[:], in0=tabb[:, 0:D], scalar=svs[j][:],
                    in1=tabb[:, D : 2 * D],
                    op0=mybir.AluOpType.mult, op1=mybir.AluOpType.add,
                )
                # f32->i32 conversion rounds to nearest even: ki = round(ang/2pi)
                nc.vector.tensor_scalar_mul(out=ki[:], in0=ang[:], scalar1=INV_2PI)
                nc.vector.scalar_tensor_tensor(
                    out=red[:], in0=ki[:], scalar=-TWO_PI, in1=ang[:],
                    op0=mybir.AluOpType.mult, op1=mybir.AluOpType.add,
                )
                nc.scalar.activation(
                    out=pe_j[:], in_=red[:], func=mybir.ActivationFunctionType.Sin
                )
                pe_tiles.append(pe_j)
                # start streaming batch 0's block j as soon as pe_j is ready
                io_block(0, j, pe_j)

            for b in range(1, B):
                for j in range(NJ):
                    io_block(b, j, pe_tiles[j])
    nc.finalize()
    return nc


_CACHE: dict = {}


def kernel(embeddings: np.ndarray) -> np.ndarray:
    global _LAST_RESULTS
    emb = np.asarray(embeddings, dtype=np.float32)
    if "nc" not in _CACHE:
        _CACHE["nc"] = _build()
        _CACHE["tab"] = _make_tab()
    nc = _CACHE["nc"]
    tab = _CACHE["tab"]

    in_maps = []
    for c in range(N_CORES):
        sl = emb[:, c * SS : (c + 1) * SS, :]
        srow = (np.arange(P, dtype=np.float32) + c * SS)[:, None]
        in_maps.append(
            {
                "emb": np.ascontiguousarray(sl),
                "srow": srow,
                "tab": tab,
            }
        )
    res = run_bass_kernel_spmd(
        nc, in_maps, core_ids=list(range(N_CORES)), trace=_TRACE
    )
    _LAST_RESULTS = res
    return np.concatenate([r["out"] for r in res.results], axis=1)
